# revision 13
# baseline (speedup 1.0000x reference)
"""2-layer GCN (Kipf) on 8 Trainium2 NeuronCores via Bass/Tile.

Sharding: nodes row-sharded 8 ways (12500/core, padded to 12544 = 98x128).
Each core computes h = x_shard @ W1 (pre-scaled by deg^-1/2), AllGathers the
transformed features, then aggregates its destination shard's edges with
dma_gather + selection-matrix matmuls (segment-sum on the PE), applies
relu / W2, AllGathers the small second-layer features, aggregates again and
finishes with log_softmax.  Edge streams are sorted by (group, src-bucket,
block) and padded to a uniform run length so the single SPMD program works
for every core; src-buckets of 2 shards keep gather indices within int16.
"""

import numpy as np
import ml_dtypes

BF16 = ml_dtypes.bfloat16

N_NODES = 100000
N_EDGES = 3200000
F_IN, F_HID, N_CLASS = 512, 256, 41

NCORE = 8
SHR = N_NODES // NCORE        # real nodes per shard (12500)
BLK = 128
NBLKG = 7                     # blocks per group (PSUM accumulators live at once)
NGRP = 14                     # groups per core
NBLK = NBLKG * NGRP           # 98 blocks
SH = NBLK * BLK               # padded shard rows (12544)
NBUK = 4                      # src buckets (2 shards each, 25088 < int16 max)
BUK = 2 * SH
NCLSP = 48                    # padded class dim
G2COL = 256                   # padded row width of layer-2 features (fp8, 256B rows)
KIN = F_IN // 128             # 4 k-slices for mm1
NQUEUE = 1                    # SWDGE queues to rotate gathers over
KH = F_HID // 128             # 2 k-slices for mm2


def _host_prep(x, edge_index, W1, b1, W2, b2):
    x = np.asarray(x, dtype=np.float32)
    src = np.asarray(edge_index[0], dtype=np.int64)
    dst = np.asarray(edge_index[1], dtype=np.int64)
    W1 = np.asarray(W1, dtype=np.float32)
    b1 = np.asarray(b1, dtype=np.float32)
    W2 = np.asarray(W2, dtype=np.float32)
    b2 = np.asarray(b2, dtype=np.float32)

    deg = (np.bincount(dst, minlength=N_NODES) + 1).astype(np.float32)
    dinv = deg ** -0.5
    # global h' row of a node (shards padded to SH rows)
    row_of = lambda n: (n // SHR) * SH + (n % SHR)

    src_row = row_of(src)
    dst_core = dst // SHR
    dst_loc = dst % SHR                      # local node within shard
    blk = dst_loc // BLK
    grp = blk // NBLKG
    blk_in_g = blk % NBLKG
    buk = src_row // BUK
    idx_in_buk = src_row % BUK

    # order edges: core, group, bucket, block-in-group, then run position
    order = np.lexsort((blk_in_g, buk, grp, dst_core))
    e_core = dst_core[order]
    e_grp = grp[order]
    e_buk = buk[order]
    e_blkg = blk_in_g[order]
    e_idx = idx_in_buk[order]
    e_dl = (dst_loc[order] % BLK).astype(np.float32)

    # run id = (core, grp, buk, blk_in_g); count lengths to find E_bb
    run_key = ((e_core * NGRP + e_grp) * NBUK + e_buk) * NBLKG + e_blkg
    nruns = NCORE * NGRP * NBUK * NBLKG
    run_len = np.bincount(run_key, minlength=nruns)
    e_bb = int(((run_len.max() + BLK - 1) // BLK) * BLK)
    e_bb = max(e_bb, BLK)

    # scatter edges into padded streams
    run_start = run_key * e_bb
    pos_in_run = np.arange(len(run_key)) - np.repeat(
        np.concatenate([[0], np.cumsum(run_len)[:-1]]), run_len
    )
    slot = run_start + pos_in_run
    idx_pad = np.zeros(nruns * e_bb, dtype=np.int16)
    dl_pad = np.full(nruns * e_bb, -1.0, dtype=np.float32)
    idx_pad[slot] = e_idx.astype(np.int16)
    dl_pad[slot] = e_dl

    # reshape [core, grp, buk, blkg*e_bb] and wrap for the device layouts
    call = NBLKG * e_bb                       # idxs per gather call
    idx_pad = idx_pad.reshape(NCORE, NGRP, NBUK, call)
    dl_pad = dl_pad.reshape(NCORE, NGRP, NBUK, call)
    # idx: element k -> [k%16, k//16], replicated to 128 partitions
    idx_dev = np.ascontiguousarray(
        np.broadcast_to(
            idx_pad.reshape(NCORE, NGRP, NBUK, call // 16, 16)
            .transpose(0, 1, 2, 4, 3)[:, :, :, None, :, :],
            (NCORE, NGRP, NBUK, 8, 16, call // 16),
        ).reshape(NCORE, NGRP, NBUK, 128, call // 16)
    )
    # dstloc: element k -> [k%128, k//128], bf16
    dl_dev = np.ascontiguousarray(
        dl_pad.reshape(NCORE, NGRP, NBUK, call // BLK, BLK).transpose(0, 1, 2, 4, 3)
    ).astype(BF16)

    # per-core dense tensors
    xp = np.zeros((NCORE, SH, F_IN), dtype=np.float32)
    xp[:, :SHR] = x.reshape(NCORE, SHR, F_IN)
    xT = np.ascontiguousarray(xp.transpose(0, 2, 1)).astype(BF16)

    dinv_p = np.zeros((NCORE, SH), dtype=np.float32)
    dinv_p[:, :SHR] = dinv.reshape(NCORE, SHR)
    dinv_dev = np.ascontiguousarray(
        dinv_p.reshape(NCORE, NBLK, BLK).transpose(0, 2, 1)
    )

    W2p = np.zeros((F_HID, NCLSP), dtype=np.float32)
    W2p[:, :N_CLASS] = W2
    b2p = np.zeros((NCLSP,), dtype=np.float32)
    b2p[:N_CLASS] = b2

    iota = np.tile(np.arange(BLK, dtype=np.float32), (BLK, 1)).astype(BF16)

    common = {
        "w1": W1.astype(BF16),
        "w2": W2p.astype(BF16),
        "b1r": b1.reshape(1, F_HID).astype(np.float32).copy(),
        "b2r": b2p.reshape(1, NCLSP).astype(np.float32).copy(),
        "iota": iota,
        "ident": np.eye(BLK, dtype=np.float32).astype(BF16),
        "ident8": np.eye(BLK, dtype=np.float32).astype(ml_dtypes.float8_e4m3),
    }
    sqd_p = np.zeros((NCORE, SH), dtype=np.float32)
    sqd_p[:, :SHR] = np.sqrt(deg).reshape(NCORE, SHR)

    in_maps = []
    for c in range(NCORE):
        m = dict(common)
        m["xt"] = xT[c]
        m["dinv"] = dinv_dev[c]
        m["sqd"] = np.ascontiguousarray(sqd_p[c].reshape(1, SH))
        m["gidx"] = idx_dev[c]
        m["gdl"] = dl_dev[c]
        in_maps.append(m)
    return in_maps, e_bb


def _build_program(e_bb, probe=False):
    from concourse import bass, bacc, mybir, tile

    f32 = mybir.dt.float32
    bf16 = mybir.dt.bfloat16
    fp8 = mybir.dt.float8e4
    i16 = mybir.dt.int16
    call = NBLKG * e_bb
    ntile = e_bb // BLK                     # tiles per run
    ctile = call // BLK                     # tiles per gather call
    GW = NBLKG * BLK                        # nodes per group

    nc = bacc.Bacc("TRN2", target_bir_lowering=False, debug=False,
                   num_devices=1 if probe else NCORE,
                   num_swdge_queues=NQUEUE)

    xt_d = nc.dram_tensor("xt", [F_IN, SH], bf16, kind="ExternalInput")
    w1_d = nc.dram_tensor("w1", [F_IN, F_HID], bf16, kind="ExternalInput")
    w2_d = nc.dram_tensor("w2", [F_HID, NCLSP], bf16, kind="ExternalInput")
    b1_d = nc.dram_tensor("b1r", [1, F_HID], f32, kind="ExternalInput")
    b2_d = nc.dram_tensor("b2r", [1, NCLSP], f32, kind="ExternalInput")
    dinv_d = nc.dram_tensor("dinv", [BLK, NBLK], f32, kind="ExternalInput")
    sqd_d = nc.dram_tensor("sqd", [1, SH], f32, kind="ExternalInput")
    gidx_d = nc.dram_tensor("gidx", [NGRP, NBUK, 128, call // 16], i16,
                            kind="ExternalInput")
    gdl_d = nc.dram_tensor("gdl", [NGRP, NBUK, BLK, ctile], bf16,
                           kind="ExternalInput")
    iota_d = nc.dram_tensor("iota", [BLK, BLK], bf16, kind="ExternalInput")
    ident_d = nc.dram_tensor("ident", [BLK, BLK], bf16, kind="ExternalInput")
    ident8_d = nc.dram_tensor("ident8", [BLK, BLK], fp8, kind="ExternalInput")
    out_d = nc.dram_tensor("out", [SH, NCLSP], f32, kind="ExternalOutput")

    rg = [list(range(NCORE))]

    with tile.TileContext(nc) as tc:
        with tc.tile_pool(name="dram", bufs=1, space="DRAM") as dram, \
             tc.tile_pool(name="const", bufs=1) as constp, \
             tc.tile_pool(name="sb", bufs=2) as sb, \
             tc.tile_pool(name="sb3", bufs=3) as sb3, \
             tc.tile_pool(name="psum", bufs=8, space="PSUM") as psum:

            h_bounce = dram.tile([SH, F_HID], fp8)
            h_full = dram.tile([NCORE * SH, F_HID], fp8,
                               addr_space="Local" if probe else "Shared")
            g_bounce = dram.tile([SH, G2COL], fp8)
            g_full = dram.tile([NCORE * SH, G2COL], fp8,
                               addr_space="Local" if probe else "Shared")

            # ---- constants ----
            w1s = constp.tile([BLK, KIN, F_HID], bf16)
            nc.sync.dma_start(out=w1s[:], in_=w1_d.ap().rearrange(
                "(a p) n -> p a n", p=BLK))
            w2s = constp.tile([BLK, KH, NCLSP], bf16)
            nc.sync.dma_start(out=w2s[:], in_=w2_d.ap().rearrange(
                "(a p) n -> p a n", p=BLK))
            b1s = constp.tile([1, F_HID], f32)
            nc.sync.dma_start(out=b1s[:], in_=b1_d.ap())
            b2s = constp.tile([1, NCLSP], f32)
            nc.sync.dma_start(out=b2s[:], in_=b2_d.ap())
            dinvs = constp.tile([BLK, NBLK], f32)
            nc.sync.dma_start(out=dinvs[:], in_=dinv_d.ap())
            iotas = constp.tile([BLK, BLK], bf16)
            nc.sync.dma_start(out=iotas[:], in_=iota_d.ap())
            idents = constp.tile([BLK, BLK], bf16)
            nc.sync.dma_start(out=idents[:], in_=ident_d.ap())
            ident8s = constp.tile([BLK, BLK], fp8)
            nc.sync.dma_start(out=ident8s[:], in_=ident8_d.ap())

            xt_r = xt_d.ap().rearrange("(a p) n -> p a n", p=BLK)
            hb_r = h_bounce[:].rearrange("(j p) f -> p j f", p=BLK)
            gb_r = g_bounce[:].rearrange("(j p) f -> p j f", p=BLK)
            out_r = out_d.ap().rearrange("(j p) f -> p j f", p=BLK)

            # ---- phase A: h' = dinv * (x @ W1), shard-local ----
            for g in range(NGRP):
                n0 = g * GW
                xg = sb.tile([BLK, KIN, GW], bf16, tag="xg")
                nc.sync.dma_start(out=xg[:], in_=xt_r[:, :, n0:n0 + GW])
                ht_g = sb.tile([BLK, NBLKG, F_HID], fp8, tag="ht")
                for j in range(NBLKG):
                    b = g * NBLKG + j
                    ps = psum.tile([BLK, F_HID], f32, tag="ps", name=f"psa{b}")
                    for k in range(KIN):
                        nc.tensor.matmul(
                            ps[:], lhsT=xg[:, k, j * BLK:(j + 1) * BLK],
                            rhs=w1s[:, k, :], start=(k == 0), stop=(k == KIN - 1))
                    nc.vector.tensor_scalar_mul(ht_g[:, j, :], ps[:],
                                                dinvs[:, b:b + 1])
                nc.sync.dma_start(
                    out=hb_r[:, g * NBLKG:(g + 1) * NBLKG, :], in_=ht_g[:])

            if not probe:
                nc.gpsimd.collective_compute(
                    "AllGather", mybir.AluOpType.bypass, replica_groups=rg,
                    ins=[h_bounce[:]], outs=[h_full[:]])

            # ---- phase B: aggregate layer 1, relu, @W2 ----
            for g in range(NGRP):
                ix_g = sb.tile([128, NBUK, call // 16], i16, tag="ix")
                nc.sync.dma_start(out=ix_g[:], in_=gidx_d.ap()[g].rearrange(
                    "b p w -> p b w"))
                dl_g = sb.tile([BLK, NBUK, ctile], bf16, tag="dl")
                nc.sync.dma_start(out=dl_g[:], in_=gdl_d.ap()[g].rearrange(
                    "b p w -> p b w"))
                hs_g = sb.tile([BLK, NBLKG, F_HID], fp8, tag="hs")
                nc.sync.dma_start(out=hs_g[:],
                                  in_=hb_r[:, g * NBLKG:(g + 1) * NBLKG, :])
                sq_g = sb.tile([1, GW], f32, tag="sq")
                nc.sync.dma_start(out=sq_g[:],
                                  in_=sqd_d.ap()[:, g * GW:(g + 1) * GW])
                pss = [psum.tile([BLK, F_HID], f32, tag="ps", name=f"psb{g}_{j}")
                       for j in range(NBLKG)]
                for bu in range(NBUK):
                    vt = sb.tile([BLK, ctile, F_HID], fp8, tag="vt")
                    nc.gpsimd.dma_gather(
                        out_ap=vt[:],
                        in_ap=h_full[bu * BUK:(bu + 1) * BUK, :],
                        idxs_ap=ix_g[:, bu, :], num_idxs=call,
                        num_idxs_reg=call, elem_size=F_HID,
                        single_packet=False,
                        queue_num=(g * NBUK + bu) % NQUEUE)
                    st = sb.tile([BLK, ctile, BLK], fp8, tag="st")
                    nc.vector.tensor_tensor(
                        out=st[:],
                        in0=iotas[:].to_broadcast([BLK, BLK, ctile]).rearrange(
                            "p k t -> p t k"),
                        in1=dl_g[:, bu, :].to_broadcast([BLK, ctile, BLK]),
                        op=mybir.AluOpType.is_equal)
                    for j in range(NBLKG):
                        for t in range(ntile):
                            tt = j * ntile + t
                            nc.tensor.matmul(
                                pss[j][:], lhsT=st[:, tt, :],
                                rhs=vt[:, tt, :],
                                start=(bu == 0 and t == 0), stop=False)
                gt_g = sb.tile([BLK, NBLKG, NCLSP], fp8, tag="gt")
                for j in range(NBLKG):
                    b = g * NBLKG + j
                    nc.tensor.matmul(pss[j][:], lhsT=ident8s[:],
                                     rhs=hs_g[:, j, :], start=False, stop=False)
                    nc.tensor.matmul(pss[j][:],
                                     lhsT=sq_g[0:1, j * BLK:(j + 1) * BLK],
                                     rhs=b1s[0:1, :], start=False, stop=True)
                    rt = sb.tile([BLK, F_HID], bf16, tag="rt")
                    nc.vector.tensor_scalar(
                        out=rt[:], in0=pss[j][:], scalar1=dinvs[:, b:b + 1],
                        scalar2=0.0, op0=mybir.AluOpType.mult,
                        op1=mybir.AluOpType.max)
                    rT = sb.tile([BLK, F_HID], bf16, tag="rT")
                    for k in range(KH):
                        pt = psum.tile([BLK, F_HID], bf16, tag="ps",
                                       name=f"pt{b}_{k}")
                        nc.tensor.transpose(pt[:, :BLK],
                                            rt[:, k * BLK:(k + 1) * BLK],
                                            idents[:])
                        nc.vector.tensor_copy(rT[:, k * BLK:(k + 1) * BLK],
                                              pt[:, :BLK])
                    ps2 = psum.tile([BLK, F_HID], f32, tag="ps", name=f"ps2_{b}")
                    for k in range(KH):
                        nc.tensor.matmul(ps2[:, :NCLSP],
                                         lhsT=rT[:, k * BLK:(k + 1) * BLK],
                                         rhs=w2s[:, k, :], start=(k == 0),
                                         stop=(k == KH - 1))
                    nc.vector.tensor_scalar_mul(gt_g[:, j, :], ps2[:, :NCLSP],
                                                dinvs[:, b:b + 1])
                nc.sync.dma_start(
                    out=gb_r[:, g * NBLKG:(g + 1) * NBLKG, :NCLSP],
                    in_=gt_g[:])

            if not probe:
                nc.gpsimd.collective_compute(
                    "AllGather", mybir.AluOpType.bypass, replica_groups=rg,
                    ins=[g_bounce[:]], outs=[g_full[:]])

            # ---- phase C: aggregate layer 2, log_softmax ----
            for g in range(NGRP):
                ix_g = sb.tile([128, NBUK, call // 16], i16, tag="ix",
                               name=f"ixc{g}")
                nc.sync.dma_start(out=ix_g[:], in_=gidx_d.ap()[g].rearrange(
                    "b p w -> p b w"))
                dl_g = sb.tile([BLK, NBUK, ctile], bf16, tag="dl",
                               name=f"dlc{g}")
                nc.sync.dma_start(out=dl_g[:], in_=gdl_d.ap()[g].rearrange(
                    "b p w -> p b w"))
                gs_g = sb.tile([BLK, NBLKG, NCLSP], fp8, tag="gs")
                nc.sync.dma_start(
                    out=gs_g[:],
                    in_=gb_r[:, g * NBLKG:(g + 1) * NBLKG, :NCLSP])
                sq_g = sb.tile([1, GW], f32, tag="sq", name=f"sqc{g}")
                nc.sync.dma_start(out=sq_g[:],
                                  in_=sqd_d.ap()[:, g * GW:(g + 1) * GW])
                pss = [psum.tile([BLK, F_HID], f32, tag="ps", name=f"psc{g}_{j}")
                       for j in range(NBLKG)]
                for bu in range(NBUK):
                    vt2 = sb.tile([BLK, ctile, G2COL], fp8, tag="vt",
                                  name=f"vtc{g}_{bu}")
                    nc.gpsimd.dma_gather(
                        out_ap=vt2[:],
                        in_ap=g_full[bu * BUK:(bu + 1) * BUK, :],
                        idxs_ap=ix_g[:, bu, :], num_idxs=call,
                        num_idxs_reg=call, elem_size=G2COL,
                        single_packet=False,
                        queue_num=(g * NBUK + bu) % NQUEUE)
                    st = sb.tile([BLK, ctile, BLK], fp8, tag="st",
                                 name=f"stc{g}_{bu}")
                    nc.vector.tensor_tensor(
                        out=st[:],
                        in0=iotas[:].to_broadcast([BLK, BLK, ctile]).rearrange(
                            "p k t -> p t k"),
                        in1=dl_g[:, bu, :].to_broadcast([BLK, ctile, BLK]),
                        op=mybir.AluOpType.is_equal)
                    for j in range(NBLKG):
                        for t in range(ntile):
                            tt = j * ntile + t
                            nc.tensor.matmul(
                                pss[j][:, :NCLSP], lhsT=st[:, tt, :],
                                rhs=vt2[:, tt, :NCLSP],
                                start=(bu == 0 and t == 0), stop=False)
                z2s, nms, ets, ses = [], [], [], []
                ot_g = sb.tile([BLK, NBLKG, N_CLASS], f32, tag="ot")
                for j in range(NBLKG):
                    b = g * NBLKG + j
                    nc.tensor.matmul(pss[j][:, :NCLSP], lhsT=ident8s[:],
                                     rhs=gs_g[:, j, :], start=False, stop=False)
                    nc.tensor.matmul(pss[j][:, :NCLSP],
                                     lhsT=sq_g[0:1, j * BLK:(j + 1) * BLK],
                                     rhs=b2s[0:1, :], start=False, stop=True)
                    z2 = sb.tile([BLK, NCLSP], f32, tag="z2", bufs=NBLKG + 1,
                                 name=f"z2_{b}")
                    nc.vector.tensor_scalar_mul(z2[:], pss[j][:, :NCLSP],
                                                dinvs[:, b:b + 1])
                    nm = sb.tile([BLK, 1], f32, tag="nm", bufs=NBLKG + 1,
                                 name=f"nm_{b}")
                    nc.vector.reduce_max(nm[:], z2[:, :N_CLASS],
                                         axis=mybir.AxisListType.X,
                                         negate=True)
                    z2s.append(z2); nms.append(nm)
                for j in range(NBLKG):
                    et = sb.tile([BLK, N_CLASS], f32, tag="et",
                                 bufs=NBLKG + 1, name=f"et_{g}_{j}")
                    se = sb.tile([BLK, 1], f32, tag="se", bufs=NBLKG + 1,
                                 name=f"se_{g}_{j}")
                    nc.scalar.activation(et[:], z2s[j][:, :N_CLASS],
                                         mybir.ActivationFunctionType.Exp,
                                         bias=nms[j][:, 0:1],
                                         accum_out=se[:])
                    ets.append(et); ses.append(se)
                for j in range(NBLKG):
                    b = g * NBLKG + j
                    ls = sb.tile([BLK, 1], f32, tag="ls", name=f"ls_{b}")
                    nc.scalar.activation(ls[:], ses[j][:],
                                         mybir.ActivationFunctionType.Ln)
                    off = sb.tile([BLK, 1], f32, tag="off", name=f"off_{b}")
                    nc.vector.tensor_sub(off[:], ls[:], nms[j][:])
                    nc.vector.tensor_scalar(
                        out=ot_g[:, j, :], in0=z2s[j][:, :N_CLASS],
                        scalar1=off[:, 0:1],
                        scalar2=None, op0=mybir.AluOpType.subtract)
                nc.sync.dma_start(
                    out=out_r[:, g * NBLKG:(g + 1) * NBLKG, :N_CLASS],
                    in_=ot_g[:])

    nc.compile()
    return nc


_CACHE = {}


def kernel(x, edge_index, W1, b1, W2, b2):
    from concourse.bass_utils import run_bass_kernel_spmd

    in_maps, e_bb = _host_prep(x, edge_index, W1, b1, W2, b2)
    nc = _CACHE.get(e_bb)
    if nc is None:
        nc = _build_program(e_bb)
        _CACHE[e_bb] = nc
    res = run_bass_kernel_spmd(nc, in_maps, core_ids=list(range(NCORE)))
    out = np.empty((N_NODES, N_CLASS), dtype=np.float32)
    for c in range(NCORE):
        out[c * SHR:(c + 1) * SHR] = res.results[c]["out"][:SHR, :N_CLASS]
    return out


# revision 14
# speedup vs baseline: 2.7190x; 2.7190x over previous
"""2-layer GCN (Kipf) on 8 Trainium2 NeuronCores via Bass/Tile.

Sharding: nodes row-sharded 8 ways (12500/core, padded to 12544 = 98x128).
Each core computes h = x_shard @ W1 (pre-scaled by deg^-1/2), AllGathers the
transformed features, then aggregates its destination shard's edges with
dma_gather + selection-matrix matmuls (segment-sum on the PE), applies
relu / W2, AllGathers the small second-layer features, aggregates again and
finishes with log_softmax.  Edge streams are sorted by (group, src-bucket,
block) and padded to a uniform run length so the single SPMD program works
for every core; src-buckets of 2 shards keep gather indices within int16.
"""

import numpy as np
import ml_dtypes

BF16 = ml_dtypes.bfloat16

N_NODES = 100000
N_EDGES = 3200000
F_IN, F_HID, N_CLASS = 512, 256, 41

NCORE = 8
SHR = N_NODES // NCORE        # real nodes per shard (12500)
BLK = 128
NBLKG = 7                     # blocks per group (PSUM accumulators live at once)
NGRP = 14                     # groups per core
NBLK = NBLKG * NGRP           # 98 blocks
SH = NBLK * BLK               # padded shard rows (12544)
NBUK = 4                      # src buckets (2 shards each, 25088 < int16 max)
BUK = 2 * SH
NCLSP = 48                    # padded class dim
G2COL = 128                   # padded row width of layer-2 features (256B rows)
KIN = F_IN // 128             # 4 k-slices for mm1
NQUEUE = 4                    # SWDGE queues to rotate gathers over
KH = F_HID // 128             # 2 k-slices for mm2


def _host_prep(x, edge_index, W1, b1, W2, b2):
    x = np.asarray(x, dtype=np.float32)
    src = np.asarray(edge_index[0], dtype=np.int64)
    dst = np.asarray(edge_index[1], dtype=np.int64)
    W1 = np.asarray(W1, dtype=np.float32)
    b1 = np.asarray(b1, dtype=np.float32)
    W2 = np.asarray(W2, dtype=np.float32)
    b2 = np.asarray(b2, dtype=np.float32)

    deg = (np.bincount(dst, minlength=N_NODES) + 1).astype(np.float32)
    dinv = deg ** -0.5
    # global h' row of a node (shards padded to SH rows)
    row_of = lambda n: (n // SHR) * SH + (n % SHR)

    src_row = row_of(src)
    dst_core = dst // SHR
    dst_loc = dst % SHR                      # local node within shard
    blk = dst_loc // BLK
    grp = blk // NBLKG
    blk_in_g = blk % NBLKG
    buk = src_row // BUK
    idx_in_buk = src_row % BUK

    # order edges: core, group, bucket, block-in-group, then run position
    order = np.lexsort((blk_in_g, buk, grp, dst_core))
    e_core = dst_core[order]
    e_grp = grp[order]
    e_buk = buk[order]
    e_blkg = blk_in_g[order]
    e_idx = idx_in_buk[order]
    e_dl = (dst_loc[order] % BLK).astype(np.float32)

    # run id = (core, grp, buk, blk_in_g); count lengths to find E_bb
    run_key = ((e_core * NGRP + e_grp) * NBUK + e_buk) * NBLKG + e_blkg
    nruns = NCORE * NGRP * NBUK * NBLKG
    run_len = np.bincount(run_key, minlength=nruns)
    e_bb = int(((run_len.max() + BLK - 1) // BLK) * BLK)
    e_bb = max(e_bb, BLK)

    # scatter edges into padded streams
    run_start = run_key * e_bb
    pos_in_run = np.arange(len(run_key)) - np.repeat(
        np.concatenate([[0], np.cumsum(run_len)[:-1]]), run_len
    )
    slot = run_start + pos_in_run
    idx_pad = np.zeros(nruns * e_bb, dtype=np.int16)
    dl_pad = np.full(nruns * e_bb, -1.0, dtype=np.float32)
    idx_pad[slot] = e_idx.astype(np.int16)
    dl_pad[slot] = e_dl

    # reshape [core, grp, buk, blkg*e_bb] and wrap for the device layouts
    call = NBLKG * e_bb                       # idxs per gather call
    idx_pad = idx_pad.reshape(NCORE, NGRP, NBUK, call)
    dl_pad = dl_pad.reshape(NCORE, NGRP, NBUK, call)
    # idx: element k -> [k%16, k//16], replicated to 128 partitions
    idx_dev = np.ascontiguousarray(
        np.broadcast_to(
            idx_pad.reshape(NCORE, NGRP, NBUK, call // 16, 16)
            .transpose(0, 1, 2, 4, 3)[:, :, :, None, :, :],
            (NCORE, NGRP, NBUK, 8, 16, call // 16),
        ).reshape(NCORE, NGRP, NBUK, 128, call // 16)
    )
    # dstloc: element k -> [k%128, k//128], bf16
    dl_dev = np.ascontiguousarray(
        dl_pad.reshape(NCORE, NGRP, NBUK, call // BLK, BLK).transpose(0, 1, 2, 4, 3)
    ).astype(BF16)

    # per-core dense tensors
    xp = np.zeros((NCORE, SH, F_IN), dtype=np.float32)
    xp[:, :SHR] = x.reshape(NCORE, SHR, F_IN)
    xT = np.ascontiguousarray(xp.transpose(0, 2, 1)).astype(BF16)

    dinv_p = np.zeros((NCORE, SH), dtype=np.float32)
    dinv_p[:, :SHR] = dinv.reshape(NCORE, SHR)
    dinv_dev = np.ascontiguousarray(
        dinv_p.reshape(NCORE, NBLK, BLK).transpose(0, 2, 1)
    )

    W2p = np.zeros((F_HID, NCLSP), dtype=np.float32)
    W2p[:, :N_CLASS] = W2
    b2p = np.zeros((NCLSP,), dtype=np.float32)
    b2p[:N_CLASS] = b2

    iota = np.tile(np.arange(BLK, dtype=np.float32), (BLK, 1)).astype(BF16)

    common = {
        "w1": W1.astype(BF16),
        "w2": W2p.astype(BF16),
        "b1r": b1.reshape(1, F_HID).astype(np.float32).copy(),
        "b2r": b2p.reshape(1, NCLSP).astype(np.float32).copy(),
        "iota": iota,
        "ident": np.eye(BLK, dtype=np.float32).astype(BF16),
        "ident8": np.eye(BLK, dtype=np.float32).astype(ml_dtypes.float8_e4m3),
    }
    sqd_p = np.zeros((NCORE, SH), dtype=np.float32)
    sqd_p[:, :SHR] = np.sqrt(deg).reshape(NCORE, SHR)

    in_maps = []
    for c in range(NCORE):
        m = dict(common)
        m["xt"] = xT[c]
        m["dinv"] = dinv_dev[c]
        m["sqd"] = np.ascontiguousarray(sqd_p[c].reshape(1, SH))
        m["gidx"] = idx_dev[c]
        m["gdl"] = dl_dev[c]
        in_maps.append(m)
    return in_maps, e_bb


def _build_program(e_bb, probe=False):
    from concourse import bass, bacc, mybir, tile

    f32 = mybir.dt.float32
    bf16 = mybir.dt.bfloat16
    fp8 = mybir.dt.float8e4
    i16 = mybir.dt.int16
    call = NBLKG * e_bb
    ntile = e_bb // BLK                     # tiles per run
    ctile = call // BLK                     # tiles per gather call
    GW = NBLKG * BLK                        # nodes per group

    nc = bacc.Bacc("TRN2", target_bir_lowering=False, debug=False,
                   num_devices=1 if probe else NCORE,
                   num_swdge_queues=NQUEUE)

    xt_d = nc.dram_tensor("xt", [F_IN, SH], bf16, kind="ExternalInput")
    w1_d = nc.dram_tensor("w1", [F_IN, F_HID], bf16, kind="ExternalInput")
    w2_d = nc.dram_tensor("w2", [F_HID, NCLSP], bf16, kind="ExternalInput")
    b1_d = nc.dram_tensor("b1r", [1, F_HID], f32, kind="ExternalInput")
    b2_d = nc.dram_tensor("b2r", [1, NCLSP], f32, kind="ExternalInput")
    dinv_d = nc.dram_tensor("dinv", [BLK, NBLK], f32, kind="ExternalInput")
    sqd_d = nc.dram_tensor("sqd", [1, SH], f32, kind="ExternalInput")
    gidx_d = nc.dram_tensor("gidx", [NGRP, NBUK, 128, call // 16], i16,
                            kind="ExternalInput")
    gdl_d = nc.dram_tensor("gdl", [NGRP, NBUK, BLK, ctile], bf16,
                           kind="ExternalInput")
    iota_d = nc.dram_tensor("iota", [BLK, BLK], bf16, kind="ExternalInput")
    ident_d = nc.dram_tensor("ident", [BLK, BLK], bf16, kind="ExternalInput")
    ident8_d = nc.dram_tensor("ident8", [BLK, BLK], fp8, kind="ExternalInput")
    out_d = nc.dram_tensor("out", [SH, NCLSP], f32, kind="ExternalOutput")

    rg = [list(range(NCORE))]

    with tile.TileContext(nc) as tc:
        with tc.tile_pool(name="dram", bufs=1, space="DRAM") as dram, \
             tc.tile_pool(name="const", bufs=1) as constp, \
             tc.tile_pool(name="sb", bufs=2) as sb, \
             tc.tile_pool(name="sb3", bufs=3) as sb3, \
             tc.tile_pool(name="psum", bufs=8, space="PSUM") as psum:

            h_bounce = dram.tile([SH, F_HID], bf16)
            h_full = dram.tile([NCORE * SH, F_HID], bf16,
                               addr_space="Local" if probe else "Shared")
            g_bounce = dram.tile([SH, G2COL], bf16)
            g_full = dram.tile([NCORE * SH, G2COL], bf16,
                               addr_space="Local" if probe else "Shared")

            # ---- constants ----
            w1s = constp.tile([BLK, KIN, F_HID], bf16)
            nc.sync.dma_start(out=w1s[:], in_=w1_d.ap().rearrange(
                "(a p) n -> p a n", p=BLK))
            w2s = constp.tile([BLK, KH, NCLSP], bf16)
            nc.sync.dma_start(out=w2s[:], in_=w2_d.ap().rearrange(
                "(a p) n -> p a n", p=BLK))
            b1s = constp.tile([1, F_HID], f32)
            nc.sync.dma_start(out=b1s[:], in_=b1_d.ap())
            b2s = constp.tile([1, NCLSP], f32)
            nc.sync.dma_start(out=b2s[:], in_=b2_d.ap())
            dinvs = constp.tile([BLK, NBLK], f32)
            nc.sync.dma_start(out=dinvs[:], in_=dinv_d.ap())
            iotas = constp.tile([BLK, BLK], bf16)
            nc.sync.dma_start(out=iotas[:], in_=iota_d.ap())
            idents = constp.tile([BLK, BLK], bf16)
            nc.sync.dma_start(out=idents[:], in_=ident_d.ap())
            ident8s = constp.tile([BLK, BLK], fp8)
            nc.sync.dma_start(out=ident8s[:], in_=ident8_d.ap())

            xt_r = xt_d.ap().rearrange("(a p) n -> p a n", p=BLK)
            hb_r = h_bounce[:].rearrange("(j p) f -> p j f", p=BLK)
            gb_r = g_bounce[:].rearrange("(j p) f -> p j f", p=BLK)
            out_r = out_d.ap().rearrange("(j p) f -> p j f", p=BLK)

            # ---- phase A: h' = dinv * (x @ W1), shard-local ----
            for g in range(NGRP):
                n0 = g * GW
                xg = sb.tile([BLK, KIN, GW], bf16, tag="xg")
                nc.sync.dma_start(out=xg[:], in_=xt_r[:, :, n0:n0 + GW])
                ht_g = sb.tile([BLK, NBLKG, F_HID], bf16, tag="ht")
                for j in range(NBLKG):
                    b = g * NBLKG + j
                    ps = psum.tile([BLK, F_HID], f32, tag="ps", name=f"psa{b}")
                    for k in range(KIN):
                        nc.tensor.matmul(
                            ps[:], lhsT=xg[:, k, j * BLK:(j + 1) * BLK],
                            rhs=w1s[:, k, :], start=(k == 0), stop=(k == KIN - 1))
                    nc.vector.tensor_scalar_mul(ht_g[:, j, :], ps[:],
                                                dinvs[:, b:b + 1])
                nc.sync.dma_start(
                    out=hb_r[:, g * NBLKG:(g + 1) * NBLKG, :], in_=ht_g[:])

            if not probe:
                nc.gpsimd.collective_compute(
                    "AllGather", mybir.AluOpType.bypass, replica_groups=rg,
                    ins=[h_bounce[:]], outs=[h_full[:]])

            # ---- phase B: aggregate layer 1, relu, @W2 ----
            for g in range(NGRP):
                ix_g = sb.tile([128, NBUK, call // 16], i16, tag="ix")
                nc.sync.dma_start(out=ix_g[:], in_=gidx_d.ap()[g].rearrange(
                    "b p w -> p b w"))
                dl_g = sb.tile([BLK, NBUK, ctile], bf16, tag="dl")
                nc.sync.dma_start(out=dl_g[:], in_=gdl_d.ap()[g].rearrange(
                    "b p w -> p b w"))
                hs_g = sb.tile([BLK, NBLKG, F_HID], bf16, tag="hs")
                nc.sync.dma_start(out=hs_g[:],
                                  in_=hb_r[:, g * NBLKG:(g + 1) * NBLKG, :])
                sq_g = sb.tile([1, GW], f32, tag="sq")
                nc.sync.dma_start(out=sq_g[:],
                                  in_=sqd_d.ap()[:, g * GW:(g + 1) * GW])
                pss = [psum.tile([BLK, F_HID], f32, tag="ps", name=f"psb{g}_{j}")
                       for j in range(NBLKG)]
                for bu in range(NBUK):
                    vt = sb.tile([BLK, ctile, F_HID], bf16, tag="vt")
                    nc.gpsimd.dma_gather(
                        out_ap=vt[:],
                        in_ap=h_full[bu * BUK:(bu + 1) * BUK, :],
                        idxs_ap=ix_g[:, bu, :], num_idxs=call,
                        num_idxs_reg=call, elem_size=F_HID,
                        single_packet=False,
                        queue_num=(g * NBUK + bu) % NQUEUE)
                    st = sb.tile([BLK, ctile, BLK], bf16, tag="st")
                    nc.vector.tensor_tensor(
                        out=st[:],
                        in0=iotas[:].to_broadcast([BLK, BLK, ctile]).rearrange(
                            "p k t -> p t k"),
                        in1=dl_g[:, bu, :].to_broadcast([BLK, ctile, BLK]),
                        op=mybir.AluOpType.is_equal)
                    for j in range(NBLKG):
                        for t in range(ntile):
                            tt = j * ntile + t
                            nc.tensor.matmul(
                                pss[j][:], lhsT=st[:, tt, :],
                                rhs=vt[:, tt, :],
                                start=(bu == 0 and t == 0), stop=False)
                gt_g = sb.tile([BLK, NBLKG, NCLSP], bf16, tag="gt")
                for j in range(NBLKG):
                    b = g * NBLKG + j
                    nc.tensor.matmul(pss[j][:], lhsT=idents[:],
                                     rhs=hs_g[:, j, :], start=False, stop=False)
                    nc.tensor.matmul(pss[j][:],
                                     lhsT=sq_g[0:1, j * BLK:(j + 1) * BLK],
                                     rhs=b1s[0:1, :], start=False, stop=True)
                    rt = sb.tile([BLK, F_HID], bf16, tag="rt")
                    nc.vector.tensor_scalar(
                        out=rt[:], in0=pss[j][:], scalar1=dinvs[:, b:b + 1],
                        scalar2=0.0, op0=mybir.AluOpType.mult,
                        op1=mybir.AluOpType.max)
                    rT = sb.tile([BLK, F_HID], bf16, tag="rT")
                    for k in range(KH):
                        pt = psum.tile([BLK, F_HID], bf16, tag="ps",
                                       name=f"pt{b}_{k}")
                        nc.tensor.transpose(pt[:, :BLK],
                                            rt[:, k * BLK:(k + 1) * BLK],
                                            idents[:])
                        nc.vector.tensor_copy(rT[:, k * BLK:(k + 1) * BLK],
                                              pt[:, :BLK])
                    ps2 = psum.tile([BLK, F_HID], f32, tag="ps", name=f"ps2_{b}")
                    for k in range(KH):
                        nc.tensor.matmul(ps2[:, :NCLSP],
                                         lhsT=rT[:, k * BLK:(k + 1) * BLK],
                                         rhs=w2s[:, k, :], start=(k == 0),
                                         stop=(k == KH - 1))
                    nc.vector.tensor_scalar_mul(gt_g[:, j, :], ps2[:, :NCLSP],
                                                dinvs[:, b:b + 1])
                nc.sync.dma_start(
                    out=gb_r[:, g * NBLKG:(g + 1) * NBLKG, :NCLSP],
                    in_=gt_g[:])

            if not probe:
                nc.gpsimd.collective_compute(
                    "AllGather", mybir.AluOpType.bypass, replica_groups=rg,
                    ins=[g_bounce[:]], outs=[g_full[:]])

            # ---- phase C: aggregate layer 2, log_softmax ----
            for g in range(NGRP):
                ix_g = sb.tile([128, NBUK, call // 16], i16, tag="ix",
                               name=f"ixc{g}")
                nc.sync.dma_start(out=ix_g[:], in_=gidx_d.ap()[g].rearrange(
                    "b p w -> p b w"))
                dl_g = sb.tile([BLK, NBUK, ctile], bf16, tag="dl",
                               name=f"dlc{g}")
                nc.sync.dma_start(out=dl_g[:], in_=gdl_d.ap()[g].rearrange(
                    "b p w -> p b w"))
                gs_g = sb.tile([BLK, NBLKG, NCLSP], bf16, tag="gs")
                nc.sync.dma_start(
                    out=gs_g[:],
                    in_=gb_r[:, g * NBLKG:(g + 1) * NBLKG, :NCLSP])
                sq_g = sb.tile([1, GW], f32, tag="sq", name=f"sqc{g}")
                nc.sync.dma_start(out=sq_g[:],
                                  in_=sqd_d.ap()[:, g * GW:(g + 1) * GW])
                pss = [psum.tile([BLK, F_HID], f32, tag="ps", name=f"psc{g}_{j}")
                       for j in range(NBLKG)]
                for bu in range(NBUK):
                    vt2 = sb.tile([BLK, ctile, G2COL], bf16, tag="vt",
                                  name=f"vtc{g}_{bu}")
                    nc.gpsimd.dma_gather(
                        out_ap=vt2[:],
                        in_ap=g_full[bu * BUK:(bu + 1) * BUK, :],
                        idxs_ap=ix_g[:, bu, :], num_idxs=call,
                        num_idxs_reg=call, elem_size=G2COL,
                        single_packet=False,
                        queue_num=(g * NBUK + bu) % NQUEUE)
                    st = sb.tile([BLK, ctile, BLK], bf16, tag="st",
                                 name=f"stc{g}_{bu}")
                    nc.vector.tensor_tensor(
                        out=st[:],
                        in0=iotas[:].to_broadcast([BLK, BLK, ctile]).rearrange(
                            "p k t -> p t k"),
                        in1=dl_g[:, bu, :].to_broadcast([BLK, ctile, BLK]),
                        op=mybir.AluOpType.is_equal)
                    for j in range(NBLKG):
                        for t in range(ntile):
                            tt = j * ntile + t
                            nc.tensor.matmul(
                                pss[j][:, :NCLSP], lhsT=st[:, tt, :],
                                rhs=vt2[:, tt, :NCLSP],
                                start=(bu == 0 and t == 0), stop=False)
                z2s, nms, ets, ses = [], [], [], []
                ot_g = sb.tile([BLK, NBLKG, N_CLASS], f32, tag="ot")
                for j in range(NBLKG):
                    b = g * NBLKG + j
                    nc.tensor.matmul(pss[j][:, :NCLSP], lhsT=idents[:],
                                     rhs=gs_g[:, j, :], start=False, stop=False)
                    nc.tensor.matmul(pss[j][:, :NCLSP],
                                     lhsT=sq_g[0:1, j * BLK:(j + 1) * BLK],
                                     rhs=b2s[0:1, :], start=False, stop=True)
                    z2 = sb.tile([BLK, NCLSP], f32, tag="z2", bufs=NBLKG + 1,
                                 name=f"z2_{b}")
                    nc.vector.tensor_scalar_mul(z2[:], pss[j][:, :NCLSP],
                                                dinvs[:, b:b + 1])
                    nm = sb.tile([BLK, 1], f32, tag="nm", bufs=NBLKG + 1,
                                 name=f"nm_{b}")
                    nc.vector.reduce_max(nm[:], z2[:, :N_CLASS],
                                         axis=mybir.AxisListType.X,
                                         negate=True)
                    z2s.append(z2); nms.append(nm)
                for j in range(NBLKG):
                    et = sb.tile([BLK, N_CLASS], f32, tag="et",
                                 bufs=NBLKG + 1, name=f"et_{g}_{j}")
                    se = sb.tile([BLK, 1], f32, tag="se", bufs=NBLKG + 1,
                                 name=f"se_{g}_{j}")
                    nc.scalar.activation(et[:], z2s[j][:, :N_CLASS],
                                         mybir.ActivationFunctionType.Exp,
                                         bias=nms[j][:, 0:1],
                                         accum_out=se[:])
                    ets.append(et); ses.append(se)
                for j in range(NBLKG):
                    b = g * NBLKG + j
                    ls = sb.tile([BLK, 1], f32, tag="ls", name=f"ls_{b}")
                    nc.scalar.activation(ls[:], ses[j][:],
                                         mybir.ActivationFunctionType.Ln)
                    off = sb.tile([BLK, 1], f32, tag="off", name=f"off_{b}")
                    nc.vector.tensor_sub(off[:], ls[:], nms[j][:])
                    nc.vector.tensor_scalar(
                        out=ot_g[:, j, :], in0=z2s[j][:, :N_CLASS],
                        scalar1=off[:, 0:1],
                        scalar2=None, op0=mybir.AluOpType.subtract)
                nc.sync.dma_start(
                    out=out_r[:, g * NBLKG:(g + 1) * NBLKG, :N_CLASS],
                    in_=ot_g[:])

    nc.compile()
    return nc


_CACHE = {}


def kernel(x, edge_index, W1, b1, W2, b2):
    from concourse.bass_utils import run_bass_kernel_spmd

    in_maps, e_bb = _host_prep(x, edge_index, W1, b1, W2, b2)
    nc = _CACHE.get(e_bb)
    if nc is None:
        nc = _build_program(e_bb)
        _CACHE[e_bb] = nc
    res = run_bass_kernel_spmd(nc, in_maps, core_ids=list(range(NCORE)))
    out = np.empty((N_NODES, N_CLASS), dtype=np.float32)
    for c in range(NCORE):
        out[c * SHR:(c + 1) * SHR] = res.results[c]["out"][:SHR, :N_CLASS]
    return out
